# revision 1
# baseline (speedup 1.0000x reference)
"""Trainium2 Bass kernel for upsample_conv_2d (conv_transpose stride-2 3x3 +
4x4 FIR + bias), data-parallel over batch on 8 NeuronCores.

Math: conv_transpose(x, w, stride 2) followed by the 4x4 FIR is a single
linear convolution with a composed 6x6 kernel on the stride-2-upsampled
grid. Phase-decomposing by output parity (h%2, w%2) turns it into FOUR
independent 3x3 same-padding convolutions on the original 64x64 grid, all
sharing the same x windows:

    out[co, 2r+pa, 2s+pb] = bias[co]
        + sum_{ci,e,f} x[ci, r+e, s+f] * K[pa,pb][co, ci, e, f]

with K[pa,pb][e,f] = G[2e-pa, 2f-pb] and
G[d1,d2] = sum_{p-u=d1, q-v=d2} w[p,q] * fir[u,v].

Each phase conv is pure channel-contraction matmul work on the PE:
lhsT = K slice [ci(128), co(128)], rhs = shifted x window [ci(128), 8h x 64w],
accumulated over 9 taps x 2 ci-blocks into PSUM, bias added on the ACT
engine during the PSUM->SBUF copy.
"""

import json

import numpy as np

import concourse.bass as bass
import concourse.mybir as mybir
import concourse.tile as tile
from concourse.bass_utils import run_bass_kernel_spmd

# ---------------------------------------------------------------------------
# BIR post-pass: this walrus build rejects instructions carrying more than one
# sem wait (e.g. Tile's kernel-tail Drain gets 3). Hoist extras into
# standalone EventSemaphore instructions right before the owner.
# ---------------------------------------------------------------------------
_MAX_WAITS = 1


def _split_waits(j: dict) -> dict:
    for fn in j.get("functions", []):
        for blk in fn.get("blocks", []):
            insts = blk.get("instructions")
            if not insts:
                continue
            out = []
            for inst in insts:
                si = inst.get("sync_info") or {}
                waits = si.get("on_wait") or []
                if len(waits) > _MAX_WAITS:
                    for k, w in enumerate(waits[_MAX_WAITS:]):
                        out.append(
                            {
                                "debug": inst.get("debug", 0),
                                "engine": inst["engine"],
                                "ins": [],
                                "name": f"{inst['name']}-wsplit{k}",
                                "opcode": "EventSemaphore",
                                "outs": [],
                                "sync_info": {"on_update": [], "on_wait": [w]},
                            }
                        )
                    si["on_wait"] = waits[:_MAX_WAITS]
                out.append(inst)
            blk["instructions"] = out
    return j


_orig_to_json_bytes = bass.Bass.to_json_bytes


def _patched_to_json_bytes(self):
    return json.dumps(_split_waits(json.loads(_orig_to_json_bytes(self)))).encode()


bass.Bass.to_json_bytes = _patched_to_json_bytes

# ---------------------------------------------------------------------------
# Problem constants (hardcoded; kernel.py must be self-contained)
# ---------------------------------------------------------------------------
N, C, H, W = 8, 256, 64, 64
OH, OW = 2 * H, 2 * W
N_CORES = 8
F32 = mybir.dt.float32
F32R = mybir.dt.float32r

# tap order shared by host weight layout and device loop
_TAPS = [(e, f, cib) for e in (-1, 0, 1) for f in (-1, 0, 1) for cib in (0, 1)]


def _phase_weight_matrix(w: np.ndarray) -> np.ndarray:
    """[256,256,3,3] conv_transpose weight -> [128, 144*128] lhsT matrix.

    Column block index = ((cib*4 + ph)*9 + tap)*2 + cob, each 128 co wide;
    row = ci within ci-block. ph = pa*2+pb, tap = (e+1)*3+(f+1).
    """
    k1 = np.array([1.0, 3.0, 3.0, 1.0], dtype=np.float64)
    fir = np.outer(k1, k1)
    fir = fir / fir.sum() * 4.0  # gain = factor^2
    wd = w.astype(np.float64)
    # G[d1+3, d2+3] = sum_{p-u=d1, q-v=d2} w[p,q] fir[u,v]
    G = np.zeros((C, C, 6, 6), dtype=np.float64)
    for p in range(3):
        for q in range(3):
            for u in range(4):
                for v in range(4):
                    G[:, :, p - u + 3, q - v + 3] += wd[:, :, p, q] * fir[u, v]
    Wmat = np.zeros((128, 2 * 4 * 9 * 2, 128), dtype=np.float32)
    for cib in range(2):
        for pa in range(2):
            for pb in range(2):
                ph = pa * 2 + pb
                for e in (-1, 0, 1):
                    for f in (-1, 0, 1):
                        tap = (e + 1) * 3 + (f + 1)
                        # K[o, c] = G[o, c, 2e-pa+3, 2f-pb+3]
                        Kof = G[:, :, 2 * e - pa + 3, 2 * f - pb + 3]
                        for cob in range(2):
                            cidx = ((cib * 4 + ph) * 9 + tap) * 2 + cob
                            blk = Kof[
                                cob * 128 : (cob + 1) * 128,
                                cib * 128 : (cib + 1) * 128,
                            ]  # [co, ci]
                            Wmat[:, cidx, :] = blk.T.astype(np.float32)
    return Wmat.reshape(128, -1)


def _widx(cib: int, ph: int, tap: int, cob: int) -> int:
    return ((cib * 4 + ph) * 9 + tap) * 2 + cob


def build_nc(reps: int = 1) -> bass.Bass:
    nc = bass.Bass("TRN2", target_bir_lowering=False, debug=False)
    x_d = nc.dram_tensor("x", [C, H + 2, W + 2], F32R, kind="ExternalInput").ap()
    w_d = nc.dram_tensor("w", [128, 144 * 128], F32R, kind="ExternalInput").ap()
    b_d = nc.dram_tensor("bias", [2, 128], F32, kind="ExternalInput").ap()
    out_d = nc.dram_tensor("out", [C, OH, OW], F32, kind="ExternalOutput").ap()

    xb = x_d.rearrange("(b p) h w -> b p h w", p=128)

    with tile.TileContext(nc) as tc:
        with (
            tc.tile_pool(name="weights", bufs=1) as wpool,
            tc.tile_pool(name="xin", bufs=1) as xpool,
            tc.tile_pool(name="psum", bufs=8, space="PSUM") as ppool,
            tc.tile_pool(name="outs", bufs=3) as opool,
        ):
            wt = wpool.tile([128, 144, 128], F32R)
            nc.sync.dma_start(wt[:], w_d.rearrange("p (a b) -> p a b", b=128))
            bt = wpool.tile([128, 2], F32)
            nc.sync.dma_start(bt[:], b_d.rearrange("b p -> p b"))

            # x arrives zero-padded to 66x66 from the host
            xpad = [xpool.tile([128, H + 2, W + 2], F32R, tag=f"xp{i}", name=f"xp{i}") for i in range(2)]
            for cib in range(2):
                nc.sync.dma_start(xpad[cib][:], xb[cib])

            for _rep in range(reps):
                for half in range(2):
                    for cob in range(2):
                        out_tiles = [
                            opool.tile([128, 8, 2, 64, 2], F32, tag="ot", name="ot")
                            for _ in range(4)
                        ]
                        for pa in range(2):
                            for pb in range(2):
                                ph = pa * 2 + pb
                                psums = [
                                    ppool.tile([128, 8, 64], F32, tag="ps", name="ps")
                                    for _ in range(4)
                                ]
                                for it, (e, f, cib) in enumerate(_TAPS):
                                    tap = (e + 1) * 3 + (f + 1)
                                    lhsT = wt[:, _widx(cib, ph, tap, cob), :]
                                    for k in range(4):
                                        hb = half * 4 + k
                                        r0 = hb * 8 + 1 + e
                                        rhs = xpad[cib][
                                            :, r0 : r0 + 8, 1 + f : 65 + f
                                        ]
                                        nc.tensor.matmul(
                                            psums[k][:],
                                            lhsT,
                                            rhs,
                                            start=(it == 0),
                                            stop=(it == len(_TAPS) - 1),
                                        )
                                for k in range(4):
                                    nc.scalar.activation(
                                        out_tiles[k][:, :, pa, :, pb],
                                        psums[k][:],
                                        mybir.ActivationFunctionType.Identity,
                                        bias=bt[:, cob : cob + 1],
                                        scale=1.0,
                                    )
                        for k in range(4):
                            hb = half * 4 + k
                            dst = out_d[
                                cob * 128 : (cob + 1) * 128,
                                hb * 16 : hb * 16 + 16,
                                :,
                            ].rearrange("c (a b) (w v) -> c a b w v", b=2, v=2)
                            nc.sync.dma_start(dst, out_tiles[k][:])
    return nc


_CACHED_NC = {}


def _get_nc(reps: int = 1) -> bass.Bass:
    if reps not in _CACHED_NC:
        _CACHED_NC[reps] = build_nc(reps)
    return _CACHED_NC[reps]


def _run(x, weight, bias, reps: int = 1):
    Wmat = _phase_weight_matrix(np.asarray(weight, dtype=np.float32))
    b2 = np.ascontiguousarray(
        np.asarray(bias, dtype=np.float32).reshape(2, 128)
    )
    xs = np.pad(
        np.asarray(x, dtype=np.float32), ((0, 0), (0, 0), (1, 1), (1, 1))
    )
    nc = _get_nc(reps)
    in_maps = [
        {"x": xs[i], "w": Wmat, "bias": b2} for i in range(N_CORES)
    ]
    res = run_bass_kernel_spmd(nc, in_maps, list(range(N_CORES)))
    return np.stack([res.results[i]["out"] for i in range(N_CORES)])


def kernel(x, weight, bias):
    return _run(x, weight, bias, reps=1)



# revision 2
# speedup vs baseline: 333.0129x; 333.0129x over previous
"""Trainium2 Bass kernel for upsample_conv_2d (conv_transpose stride-2 3x3 +
4x4 FIR + bias), data-parallel over batch on 8 NeuronCores.

Algorithm (per core = one batch image):

Stage 1 (PE): phase-decomposed conv_transpose. y[2R+pa, 2S+pb] =
  sum_{i,j,ci} w[pa+2i, pb+2j][ci,co] * x[ci, R-1+pa+i, S-1+pb+j]
-> 9 channel-contraction taps total across the 4 phases (vs 36 for the
fully-composed kernel). Weights are pre-scaled by 1/16 (the FIR per-axis
1/4 gains) and bias/64 is folded in during the PSUM->SBUF drain (ACT),
which also casts to bf16. Phase tiles Yp[pa] are [128, 66, 132] with the
two column phases packed side by side and a bias/64 pad frame so the FIR
boundary handling is exact.

Stage 2 (GpSimd + DVE): the 4x4 FIR = outer((1,3,3,1),(1,3,3,1))/16 on the
2x-upsampled grid, evaluated as three box-filter adds per axis directly in
phase space (bf16, DVE 2x mode), in chunks of 16 output rows:
  C1[m] = y[m] + y[m+1]; C2[m] = C1[m] + C1[m+1]; V[A] = C2[A-1] + C2[A]
then the same cascade over columns; the final add writes fp32 directly
into the interleaved output staging tile.

Issue order: stage-1 iterates row-groups outer / phases inner (edge strips
first) so stage-2 chunks become runnable early; stage-2(cob=0) is emitted
interleaved with stage-1(cob=1) to keep all engines busy.
"""

import json

import numpy as np

import concourse.bass as bass
import concourse.mybir as mybir
import concourse.tile as tile
from concourse.bass_utils import run_bass_kernel_spmd

# ---------------------------------------------------------------------------
# BIR post-pass: this walrus build rejects instructions carrying more than one
# sem wait (e.g. Tile's kernel-tail Drain gets 3). Hoist extras into
# standalone EventSemaphore instructions right before the owner.
# ---------------------------------------------------------------------------
_MAX_WAITS = 1


def _split_waits(j: dict) -> dict:
    for fn in j.get("functions", []):
        for blk in fn.get("blocks", []):
            insts = blk.get("instructions")
            if not insts:
                continue
            out = []
            for inst in insts:
                si = inst.get("sync_info") or {}
                waits = si.get("on_wait") or []
                if len(waits) > _MAX_WAITS:
                    for k, w in enumerate(waits[_MAX_WAITS:]):
                        out.append(
                            {
                                "debug": inst.get("debug", 0),
                                "engine": inst["engine"],
                                "ins": [],
                                "name": f"{inst['name']}-wsplit{k}",
                                "opcode": "EventSemaphore",
                                "outs": [],
                                "sync_info": {"on_update": [], "on_wait": [w]},
                            }
                        )
                    si["on_wait"] = waits[:_MAX_WAITS]
                out.append(inst)
            blk["instructions"] = out
    return j


_orig_to_json_bytes = bass.Bass.to_json_bytes


def _patched_to_json_bytes(self):
    return json.dumps(_split_waits(json.loads(_orig_to_json_bytes(self)))).encode()


bass.Bass.to_json_bytes = _patched_to_json_bytes

# ---------------------------------------------------------------------------
# Problem constants (hardcoded; kernel.py must be self-contained)
# ---------------------------------------------------------------------------
N, C, H, W = 8, 256, 64, 64
OH, OW = 2 * H, 2 * W
N_CORES = 8
F32 = mybir.dt.float32
F32R = mybir.dt.float32r
BF16 = mybir.dt.bfloat16
IDENT = mybir.ActivationFunctionType.Identity

_PHASES = [(0, 0), (0, 1), (1, 0), (1, 1)]


def _taps(pa, pb):
    ii = (0, 1) if pa == 0 else (0,)
    jj = (0, 1) if pb == 0 else (0,)
    return [(i, j) for i in ii for j in jj]


_WBLOCKS = []
for pa, pb in _PHASES:
    for i, j in _taps(pa, pb):
        for cib in range(2):
            for cob in range(2):
                _WBLOCKS.append((pa, pb, i, j, cib, cob))
_WIDX = {k: n for n, k in enumerate(_WBLOCKS)}
NW = len(_WBLOCKS)  # 36


def _stage1_weights(w: np.ndarray) -> np.ndarray:
    """[256,256,3,3] -> lhsT [128 ci, NW, 128 co], scaled by 1/16."""
    Wm = np.zeros((128, NW, 128), dtype=np.float32)
    for n, (pa, pb, i, j, cib, cob) in enumerate(_WBLOCKS):
        blk = w[
            cob * 128 : (cob + 1) * 128, cib * 128 : (cib + 1) * 128, pa + 2 * i, pb + 2 * j
        ]  # [co, ci]
        Wm[:, n, :] = blk.T / 16.0
    return Wm


def build_nc(reps: int = 1) -> bass.Bass:
    nc = bass.Bass("TRN2", target_bir_lowering=False, debug=False)
    x_d = nc.dram_tensor("x", [C, H + 2, W + 2], BF16, kind="ExternalInput").ap()
    w_d = nc.dram_tensor("w", [128, NW * 128], BF16, kind="ExternalInput").ap()
    b_d = nc.dram_tensor("bias", [2, 128], F32, kind="ExternalInput").ap()
    out_d = nc.dram_tensor("out", [C, OH, OW], F32, kind="ExternalOutput").ap()

    xb = x_d.rearrange("(b p) h w -> b p h w", p=128)
    wb = w_d.rearrange("p (a b) -> p a b", b=128)

    with tile.TileContext(nc) as tc:
        with (
            tc.tile_pool(name="const", bufs=1) as cpool,
            tc.tile_pool(name="ypers", bufs=1) as ypool,
            tc.tile_pool(name="psum", bufs=3, space="PSUM") as ppool,
            tc.tile_pool(name="pedge", bufs=2, space="PSUM") as epool,
            tc.tile_pool(name="s2", bufs=3) as spool,
            tc.tile_pool(name="outs", bufs=3) as opool,
        ):
            # split input DMAs into bands so PE can start early
            wt = cpool.tile([128, NW, 128], BF16)
            for h in range(2):
                nc.sync.dma_start(wt[:, h * 18 : h * 18 + 18, :], wb[:, h * 18 : h * 18 + 18, :])
            bt = cpool.tile([128, 2], F32)
            nc.sync.dma_start(bt[:], b_d.rearrange("b p -> p b"))
            zt = cpool.tile([128, 132], F32)
            nc.vector.memset(zt[:], 0.0)

            xpad = [cpool.tile([128, H + 2, W + 2], BF16, name=f"xp{i}") for i in range(2)]
            for cib in range(2):
                for r0, r1 in ((0, 24), (24, 48), (48, 66)):
                    nc.sync.dma_start(
                        xpad[cib][:, r0:r1, :], xb[cib][:, r0:r1, :]
                    )

            # persistent Y phase tiles, frames pre-filled with bias/64
            Yp = {}
            for cob in range(2):
                for pa in range(2):
                    t = ypool.tile([128, 66, 132], BF16, name=f"Y{cob}{pa}")
                    Yp[(cob, pa)] = t
                    bias_ap = bt[:, cob : cob + 1]
                    frame_rows = [65] if pa == 0 else [0, 65]
                    for fr in frame_rows:
                        nc.scalar.activation(
                            t[:, fr, :], zt[:], IDENT, bias=bias_ap, scale=1.0
                        )
                    for fc in (65, 66, 131):
                        nc.scalar.activation(
                            t[:, :, fc], zt[:, 0:66], IDENT, bias=bias_ap, scale=1.0
                        )

            def s1_edges(cob):
                """Edge col strips (S=64 for pb=0 phases) + row remainders
                (R=64 for pa=0 phases), all accumulated in one psum bank."""
                bias_ap = bt[:, cob : cob + 1]
                pe = epool.tile([128, 512], F32, tag="pe", name="pe")
                off = 0
                drains = []
                for pa, pb in _PHASES:
                    taps = _taps(pa, pb)
                    nR = 65 if pa == 0 else 64
                    t0 = 0 if pa == 0 else 1
                    u0 = 0 if pb == 0 else 67
                    yt = Yp[(cob, pa)]
                    acc = [(i, j, cib) for (i, j) in taps for cib in range(2)]
                    if pb == 0:  # col strip S=64, rows 0..nR-1
                        for st, (i, j, cib) in enumerate(acc):
                            lhsT = wt[:, _WIDX[(pa, pb, i, j, cib, cob)], :]
                            rhs = xpad[cib][:, pa + i : pa + i + nR, 64 + pb + j]
                            nc.tensor.matmul(
                                pe[:, off : off + nR],
                                lhsT,
                                rhs,
                                start=(st == 0),
                                stop=(st == len(acc) - 1),
                            )
                        drains.append((yt[:, t0 : t0 + nR, u0 + 64], pe[:, off : off + nR]))
                        off += nR
                    if pa == 0:  # row remainder R=64, cols 0..63
                        for st, (i, j, cib) in enumerate(acc):
                            lhsT = wt[:, _WIDX[(pa, pb, i, j, cib, cob)], :]
                            rhs = xpad[cib][:, 64 + pa + i, pb + j : pb + j + 64]
                            nc.tensor.matmul(
                                pe[:, off : off + 64],
                                lhsT,
                                rhs,
                                start=(st == 0),
                                stop=(st == len(acc) - 1),
                            )
                        drains.append((yt[:, t0 + 64, u0 : u0 + 64], pe[:, off : off + 64]))
                        off += 64
                for dst, src in drains:
                    nc.scalar.activation(dst, src, IDENT, bias=bias_ap, scale=1.0)

            def s1_rowgroup(cob, rg):
                """Main-grid rows rg*16..rg*16+15, cols 0..63, all 4 phases."""
                bias_ap = bt[:, cob : cob + 1]
                R0 = rg * 16
                for pa, pb in _PHASES:
                    taps = _taps(pa, pb)
                    t0 = 0 if pa == 0 else 1
                    u0 = 0 if pb == 0 else 67
                    yt = Yp[(cob, pa)]
                    ps = ppool.tile([128, 16, 64], F32, tag="ps", name="ps")
                    for sub in range(2):
                        Rs = R0 + sub * 8
                        acc = [(i, j, cib) for (i, j) in taps for cib in range(2)]
                        for st, (i, j, cib) in enumerate(acc):
                            lhsT = wt[:, _WIDX[(pa, pb, i, j, cib, cob)], :]
                            rhs = xpad[cib][
                                :, Rs + pa + i : Rs + pa + i + 8, pb + j : pb + j + 64
                            ]
                            nc.tensor.matmul(
                                ps[:, sub * 8 : sub * 8 + 8, :],
                                lhsT,
                                rhs,
                                start=(st == 0),
                                stop=(st == len(acc) - 1),
                            )
                    nc.scalar.activation(
                        yt[:, t0 + R0 : t0 + R0 + 16, u0 : u0 + 64],
                        ps[:],
                        IDENT,
                        bias=bias_ap,
                        scale=1.0,
                    )

            def s2_chunk(cob, k):
                """16 output rows 16k..16k+15."""
                y0 = Yp[(cob, 0)]
                y1 = Yp[(cob, 1)]
                r = 8 * k
                c1e = spool.tile([128, 9, 132], BF16, tag="c1e", name="c1e")
                c1o = spool.tile([128, 9, 132], BF16, tag="c1o", name="c1o")
                nc.gpsimd.tensor_add(c1e[:], y0[:, r : r + 9, :], y1[:, r + 1 : r + 10, :])
                nc.gpsimd.tensor_add(c1o[:], y1[:, r : r + 9, :], y0[:, r : r + 9, :])
                c2e = spool.tile([128, 8, 132], BF16, tag="c2e", name="c2e")
                c2o = spool.tile([128, 9, 132], BF16, tag="c2o", name="c2o")
                nc.vector.tensor_add(c2e[:], c1e[:, 0:8, :], c1o[:, 1:9, :])
                nc.vector.tensor_add(c2o[:], c1o[:], c1e[:])
                # V packed: rows 0..7 = even out rows (ra=0), 8..15 = odd
                vv = spool.tile([128, 16, 132], BF16, tag="vv", name="vv")
                nc.vector.tensor_add(vv[:, 0:8, :], c2o[:, 0:8, :], c2e[:])
                nc.vector.tensor_add(vv[:, 8:16, :], c2e[:], c2o[:, 1:9, :])
                d1e = spool.tile([128, 16, 65], BF16, tag="d1e", name="d1e")
                d1o = spool.tile([128, 16, 65], BF16, tag="d1o", name="d1o")
                nc.vector.tensor_add(d1e[:], vv[:, :, 0:65], vv[:, :, 67:132])
                nc.vector.tensor_add(d1o[:], vv[:, :, 66:131], vv[:, :, 0:65])
                d2e = spool.tile([128, 16, 64], BF16, tag="d2e", name="d2e")
                d2o = spool.tile([128, 16, 65], BF16, tag="d2o", name="d2o")
                nc.vector.tensor_add(d2e[:], d1e[:, :, 0:64], d1o[:, :, 1:65])
                nc.gpsimd.tensor_add(d2o[:], d1o[:], d1e[:])
                og = opool.tile([128, 16, 128], F32, tag="og", name="og")
                ov = og.rearrange("p (q a) (s b) -> p q a s b", a=2, b=2)
                for ra in range(2):
                    rs = slice(ra * 8, ra * 8 + 8)
                    nc.gpsimd.tensor_add(ov[:, :, ra, :, 0], d2o[:, rs, 0:64], d2e[:, rs, :])
                    nc.gpsimd.tensor_add(ov[:, :, ra, :, 1], d2e[:, rs, :], d2o[:, rs, 1:65])
                dst = out_d[cob * 128 : (cob + 1) * 128, 16 * k : 16 * k + 16, :]
                nc.sync.dma_start(dst, og[:])

            def body():
                # stage 1 cob=0
                s1_edges(0)
                for rg in range(4):
                    s1_rowgroup(0, rg)
                # interleave stage-2(0) with stage-1(1)
                s1_edges(1)
                for rg in range(4):
                    s1_rowgroup(1, rg)
                    s2_chunk(0, 2 * rg)
                    s2_chunk(0, 2 * rg + 1)
                for k in range(8):
                    s2_chunk(1, k)

            if reps == 1:
                body()
            else:
                with tc.For_i(0, reps):
                    body()
    return nc


_CACHED_NC = {}


def _get_nc(reps: int = 1) -> bass.Bass:
    if reps not in _CACHED_NC:
        _CACHED_NC[reps] = build_nc(reps)
    return _CACHED_NC[reps]


def _prep(x, weight, bias):
    import ml_dtypes

    Wm = _stage1_weights(np.asarray(weight, dtype=np.float32))
    b2 = np.ascontiguousarray(
        (np.asarray(bias, dtype=np.float32) / 64.0).reshape(2, 128)
    )
    xs = np.pad(
        np.asarray(x, dtype=np.float32), ((0, 0), (0, 0), (1, 1), (1, 1))
    )
    return (
        xs.astype(ml_dtypes.bfloat16),
        Wm.reshape(128, -1).astype(ml_dtypes.bfloat16),
        b2,
    )


def _run(x, weight, bias, reps: int = 1):
    xs, Wm, b2 = _prep(x, weight, bias)
    nc = _get_nc(reps)
    in_maps = [{"x": xs[i], "w": Wm, "bias": b2} for i in range(N_CORES)]
    res = run_bass_kernel_spmd(nc, in_maps, list(range(N_CORES)))
    return np.stack([res.results[i]["out"] for i in range(N_CORES)])


def kernel(x, weight, bias):
    return _run(x, weight, bias, reps=1)


# revision 3
# speedup vs baseline: 359.7582x; 1.0803x over previous
"""Trainium2 Bass kernel for upsample_conv_2d (conv_transpose stride-2 3x3 +
4x4 FIR + bias), data-parallel over batch on 8 NeuronCores.

Algorithm (per core = one batch image):

Stage 1 (PE): phase-decomposed conv_transpose. y[2R+pa, 2S+pb] =
  sum_{i,j,ci} w[pa+2i, pb+2j][ci,co] * x[ci, R-1+pa+i, S-1+pb+j]
-> 9 channel-contraction taps total across the 4 phases (vs 36 for the
fully-composed kernel). Weights are pre-scaled by 1/16 (the FIR per-axis
1/4 gains) and bias/64 is folded in during the PSUM->SBUF drain (ACT),
which also casts to bf16. Phase tiles Yp[pa] are [128, 66, 132] with the
two column phases packed side by side and a bias/64 pad frame so the FIR
boundary handling is exact.

Stage 2 (GpSimd + DVE): the 4x4 FIR = outer((1,3,3,1),(1,3,3,1))/16 on the
2x-upsampled grid, evaluated as three box-filter adds per axis directly in
phase space (bf16, DVE 2x mode), in chunks of 16 output rows:
  C1[m] = y[m] + y[m+1]; C2[m] = C1[m] + C1[m+1]; V[A] = C2[A-1] + C2[A]
then the same cascade over columns; the final add writes fp32 directly
into the interleaved output staging tile.

Issue order: stage-1 iterates row-groups outer / phases inner (edge strips
first) so stage-2 chunks become runnable early; stage-2(cob=0) is emitted
interleaved with stage-1(cob=1) to keep all engines busy.
"""

import json

import numpy as np

import concourse.bass as bass
import concourse.mybir as mybir
import concourse.tile as tile
from concourse.bass_utils import run_bass_kernel_spmd

# ---------------------------------------------------------------------------
# BIR post-pass: this walrus build rejects instructions carrying more than one
# sem wait (e.g. Tile's kernel-tail Drain gets 3). Hoist extras into
# standalone EventSemaphore instructions right before the owner.
# ---------------------------------------------------------------------------
_MAX_WAITS = 1


def _split_waits(j: dict) -> dict:
    for fn in j.get("functions", []):
        for blk in fn.get("blocks", []):
            insts = blk.get("instructions")
            if not insts:
                continue
            out = []
            for inst in insts:
                si = inst.get("sync_info") or {}
                waits = si.get("on_wait") or []
                if len(waits) > _MAX_WAITS:
                    for k, w in enumerate(waits[_MAX_WAITS:]):
                        out.append(
                            {
                                "debug": inst.get("debug", 0),
                                "engine": inst["engine"],
                                "ins": [],
                                "name": f"{inst['name']}-wsplit{k}",
                                "opcode": "EventSemaphore",
                                "outs": [],
                                "sync_info": {"on_update": [], "on_wait": [w]},
                            }
                        )
                    si["on_wait"] = waits[:_MAX_WAITS]
                out.append(inst)
            blk["instructions"] = out
    return j


_orig_to_json_bytes = bass.Bass.to_json_bytes


def _patched_to_json_bytes(self):
    return json.dumps(_split_waits(json.loads(_orig_to_json_bytes(self)))).encode()


bass.Bass.to_json_bytes = _patched_to_json_bytes

# ---------------------------------------------------------------------------
# Problem constants (hardcoded; kernel.py must be self-contained)
# ---------------------------------------------------------------------------
N, C, H, W = 8, 256, 64, 64
OH, OW = 2 * H, 2 * W
N_CORES = 8
F32 = mybir.dt.float32
F32R = mybir.dt.float32r
BF16 = mybir.dt.bfloat16
IDENT = mybir.ActivationFunctionType.Identity

_PHASES = [(0, 0), (0, 1), (1, 0), (1, 1)]


def _taps(pa, pb):
    ii = (0, 1) if pa == 0 else (0,)
    jj = (0, 1) if pb == 0 else (0,)
    return [(i, j) for i in ii for j in jj]


_WBLOCKS = []
for pa, pb in _PHASES:
    for i, j in _taps(pa, pb):
        for cib in range(2):
            for cob in range(2):
                _WBLOCKS.append((pa, pb, i, j, cib, cob))
_WIDX = {k: n for n, k in enumerate(_WBLOCKS)}
NW = len(_WBLOCKS)  # 36


def _stage1_weights(w: np.ndarray) -> np.ndarray:
    """[256,256,3,3] -> lhsT [128 ci, NW, 128 co], scaled by 1/16."""
    Wm = np.zeros((128, NW, 128), dtype=np.float32)
    for n, (pa, pb, i, j, cib, cob) in enumerate(_WBLOCKS):
        blk = w[
            cob * 128 : (cob + 1) * 128, cib * 128 : (cib + 1) * 128, pa + 2 * i, pb + 2 * j
        ]  # [co, ci]
        Wm[:, n, :] = blk.T / 16.0
    return Wm


def build_nc(reps: int = 1) -> bass.Bass:
    nc = bass.Bass("TRN2", target_bir_lowering=False, debug=False)
    x_d = nc.dram_tensor("x", [C, H + 2, W + 2], BF16, kind="ExternalInput").ap()
    w_d = nc.dram_tensor("w", [128, NW * 128], BF16, kind="ExternalInput").ap()
    b_d = nc.dram_tensor("bias", [2, 128], F32, kind="ExternalInput").ap()
    out_d = nc.dram_tensor("out", [C, OH, OW], F32, kind="ExternalOutput").ap()

    xb = x_d.rearrange("(b p) h w -> b p h w", p=128)
    wb = w_d.rearrange("p (a b) -> p a b", b=128)

    with tile.TileContext(nc) as tc:
        with (
            tc.tile_pool(name="const", bufs=1) as cpool,
            tc.tile_pool(name="ypers", bufs=1) as ypool,
            tc.tile_pool(name="psum", bufs=3, space="PSUM") as ppool,
            tc.tile_pool(name="pedge", bufs=2, space="PSUM") as epool,
            tc.tile_pool(name="s2", bufs=3) as spool,
            tc.tile_pool(name="outs", bufs=3) as opool,
        ):
            # split input DMAs into bands so PE can start early
            wt = cpool.tile([128, NW, 128], BF16)
            for h in range(2):
                nc.sync.dma_start(wt[:, h * 18 : h * 18 + 18, :], wb[:, h * 18 : h * 18 + 18, :])
            bt = cpool.tile([128, 2], F32)
            nc.sync.dma_start(bt[:], b_d.rearrange("b p -> p b"))
            zt = cpool.tile([128, 132], F32)
            nc.vector.memset(zt[:], 0.0)

            xpad = [cpool.tile([128, H + 2, W + 2], BF16, name=f"xp{i}") for i in range(2)]
            for cib in range(2):
                for r0, r1 in ((0, 24), (24, 48), (48, 66)):
                    nc.sync.dma_start(
                        xpad[cib][:, r0:r1, :], xb[cib][:, r0:r1, :]
                    )

            # persistent Y phase tiles, frames pre-filled with bias/64
            Yp = {}
            for cob in range(2):
                for pa in range(2):
                    t = ypool.tile([128, 66, 132], BF16, name=f"Y{cob}{pa}")
                    Yp[(cob, pa)] = t
                    bias_ap = bt[:, cob : cob + 1]
                    frame_rows = [65] if pa == 0 else [0, 65]
                    for fr in frame_rows:
                        nc.scalar.activation(
                            t[:, fr, :], zt[:], IDENT, bias=bias_ap, scale=1.0
                        )
                    for fc in (65, 66, 131):
                        nc.scalar.activation(
                            t[:, :, fc], zt[:, 0:66], IDENT, bias=bias_ap, scale=1.0
                        )

            def s1_edges(cob):
                """Edge col strips (S=64 for pb=0 phases) + row remainders
                (R=64 for pa=0 phases), all accumulated in one psum bank."""
                bias_ap = bt[:, cob : cob + 1]
                pe = epool.tile([128, 512], F32, tag="pe", name="pe")
                off = 0
                drains = []
                for pa, pb in _PHASES:
                    taps = _taps(pa, pb)
                    nR = 65 if pa == 0 else 64
                    t0 = 0 if pa == 0 else 1
                    u0 = 0 if pb == 0 else 67
                    yt = Yp[(cob, pa)]
                    acc = [(i, j, cib) for (i, j) in taps for cib in range(2)]
                    if pb == 0:  # col strip S=64, rows 0..nR-1
                        for st, (i, j, cib) in enumerate(acc):
                            lhsT = wt[:, _WIDX[(pa, pb, i, j, cib, cob)], :]
                            rhs = xpad[cib][:, pa + i : pa + i + nR, 64 + pb + j]
                            nc.tensor.matmul(
                                pe[:, off : off + nR],
                                lhsT,
                                rhs,
                                start=(st == 0),
                                stop=(st == len(acc) - 1),
                            )
                        drains.append((yt[:, t0 : t0 + nR, u0 + 64], pe[:, off : off + nR]))
                        off += nR
                    if pa == 0:  # row remainder R=64, cols 0..63
                        for st, (i, j, cib) in enumerate(acc):
                            lhsT = wt[:, _WIDX[(pa, pb, i, j, cib, cob)], :]
                            rhs = xpad[cib][:, 64 + pa + i, pb + j : pb + j + 64]
                            nc.tensor.matmul(
                                pe[:, off : off + 64],
                                lhsT,
                                rhs,
                                start=(st == 0),
                                stop=(st == len(acc) - 1),
                            )
                        drains.append((yt[:, t0 + 64, u0 : u0 + 64], pe[:, off : off + 64]))
                        off += 64
                for dst, src in drains:
                    nc.scalar.activation(dst, src, IDENT, bias=bias_ap, scale=1.0)

            def s1_rowgroup(cob, rg):
                """Main-grid rows rg*16..rg*16+15, cols 0..63, all 4 phases."""
                bias_ap = bt[:, cob : cob + 1]
                R0 = rg * 16
                for pa, pb in _PHASES:
                    taps = _taps(pa, pb)
                    t0 = 0 if pa == 0 else 1
                    u0 = 0 if pb == 0 else 67
                    yt = Yp[(cob, pa)]
                    ps = ppool.tile([128, 16, 64], F32, tag="ps", name="ps")
                    for sub in range(2):
                        Rs = R0 + sub * 8
                        acc = [(i, j, cib) for (i, j) in taps for cib in range(2)]
                        for st, (i, j, cib) in enumerate(acc):
                            lhsT = wt[:, _WIDX[(pa, pb, i, j, cib, cob)], :]
                            rhs = xpad[cib][
                                :, Rs + pa + i : Rs + pa + i + 8, pb + j : pb + j + 64
                            ]
                            nc.tensor.matmul(
                                ps[:, sub * 8 : sub * 8 + 8, :],
                                lhsT,
                                rhs,
                                start=(st == 0),
                                stop=(st == len(acc) - 1),
                            )
                    nc.scalar.activation(
                        yt[:, t0 + R0 : t0 + R0 + 16, u0 : u0 + 64],
                        ps[:],
                        IDENT,
                        bias=bias_ap,
                        scale=1.0,
                    )

            def s2_chunk(cob, k):
                """16 output rows 16k..16k+15."""
                y0 = Yp[(cob, 0)]
                y1 = Yp[(cob, 1)]
                r = 8 * k
                c1e = spool.tile([128, 9, 132], BF16, tag="c1e", name="c1e")
                c1o = spool.tile([128, 9, 132], BF16, tag="c1o", name="c1o")
                nc.gpsimd.tensor_add(c1e[:], y0[:, r : r + 9, :], y1[:, r + 1 : r + 10, :])
                nc.gpsimd.tensor_add(c1o[:], y1[:, r : r + 9, :], y0[:, r : r + 9, :])
                c2e = spool.tile([128, 8, 132], BF16, tag="c2e", name="c2e")
                c2o = spool.tile([128, 9, 132], BF16, tag="c2o", name="c2o")
                nc.vector.tensor_add(c2e[:], c1e[:, 0:8, :], c1o[:, 1:9, :])
                nc.vector.tensor_add(c2o[:], c1o[:], c1e[:])
                # V packed: rows 0..7 = even out rows (ra=0), 8..15 = odd
                vv = spool.tile([128, 16, 132], BF16, tag="vv", name="vv")
                nc.vector.tensor_add(vv[:, 0:8, :], c2o[:, 0:8, :], c2e[:])
                nc.vector.tensor_add(vv[:, 8:16, :], c2e[:], c2o[:, 1:9, :])
                d1e = spool.tile([128, 16, 65], BF16, tag="d1e", name="d1e")
                d1o = spool.tile([128, 16, 65], BF16, tag="d1o", name="d1o")
                nc.vector.tensor_add(d1e[:], vv[:, :, 0:65], vv[:, :, 67:132])
                nc.vector.tensor_add(d1o[:], vv[:, :, 66:131], vv[:, :, 0:65])
                d2e = spool.tile([128, 16, 64], BF16, tag="d2e", name="d2e")
                d2o = spool.tile([128, 16, 65], BF16, tag="d2o", name="d2o")
                nc.vector.tensor_add(d2e[:], d1e[:, :, 0:64], d1o[:, :, 1:65])
                # GpSimd is ~3x slower than DVE bf16 on HW: give it only a
                # balanced share (C1 always, D2o on even chunks).
                d2o_eng = nc.gpsimd if k % 2 == 0 else nc.vector
                d2o_eng.tensor_add(d2o[:], d1o[:], d1e[:])
                # finals in bf16 on DVE (2x mode), ACT does the fp32
                # interleave-cast into the DMA staging tile
                ob = spool.tile([128, 8, 2, 2, 64], BF16, tag="ob", name="ob")
                for ra in range(2):
                    rs = slice(ra * 8, ra * 8 + 8)
                    nc.vector.tensor_add(ob[:, :, ra, 0, :], d2o[:, rs, 0:64], d2e[:, rs, :])
                    nc.vector.tensor_add(ob[:, :, ra, 1, :], d2e[:, rs, :], d2o[:, rs, 1:65])
                og = opool.tile([128, 16, 128], F32, tag="og", name="og")
                ov = og.rearrange("p (q a) (s b) -> p q a s b", a=2, b=2)
                ovp = ov.rearrange("p q a s b -> p q a b s")
                nc.scalar.copy(ovp[:], ob[:])
                dst = out_d[cob * 128 : (cob + 1) * 128, 16 * k : 16 * k + 16, :]
                nc.sync.dma_start(dst, og[:])

            def body():
                # interleave: emit each stage-2 chunk right after the last
                # stage-1 row-group its Y rows depend on has been issued.
                for cob in range(2):
                    s1_edges(cob)
                    s1_rowgroup(cob, 0)
                    s1_rowgroup(cob, 1)
                    s2_chunk(cob, 0)
                    s2_chunk(cob, 1)
                    s2_chunk(cob, 2)
                    s1_rowgroup(cob, 2)
                    s2_chunk(cob, 3)
                    s2_chunk(cob, 4)
                    s1_rowgroup(cob, 3)
                    s2_chunk(cob, 5)
                    s2_chunk(cob, 6)
                    s2_chunk(cob, 7)

            if reps == 1:
                body()
            else:
                with tc.For_i(0, reps):
                    body()
    return nc


_CACHED_NC = {}


def _get_nc(reps: int = 1) -> bass.Bass:
    if reps not in _CACHED_NC:
        _CACHED_NC[reps] = build_nc(reps)
    return _CACHED_NC[reps]


def _prep(x, weight, bias):
    import ml_dtypes

    Wm = _stage1_weights(np.asarray(weight, dtype=np.float32))
    b2 = np.ascontiguousarray(
        (np.asarray(bias, dtype=np.float32) / 64.0).reshape(2, 128)
    )
    xs = np.pad(
        np.asarray(x, dtype=np.float32), ((0, 0), (0, 0), (1, 1), (1, 1))
    )
    return (
        xs.astype(ml_dtypes.bfloat16),
        Wm.reshape(128, -1).astype(ml_dtypes.bfloat16),
        b2,
    )


def _run(x, weight, bias, reps: int = 1):
    xs, Wm, b2 = _prep(x, weight, bias)
    nc = _get_nc(reps)
    in_maps = [{"x": xs[i], "w": Wm, "bias": b2} for i in range(N_CORES)]
    res = run_bass_kernel_spmd(nc, in_maps, list(range(N_CORES)))
    return np.stack([res.results[i]["out"] for i in range(N_CORES)])


def kernel(x, weight, bias):
    return _run(x, weight, bias, reps=1)


# revision 4
# speedup vs baseline: 563.5263x; 1.5664x over previous
"""Trainium2 Bass kernel for upsample_conv_2d (conv_transpose stride-2 3x3 +
4x4 FIR + bias), data-parallel over batch on 8 NeuronCores.

Algorithm (per core = one batch image):

Stage 1 (PE): phase-decomposed conv_transpose. y[2R+pa, 2S+pb] =
  sum_{i,j,ci} w[pa+2i, pb+2j][ci,co] * x[ci, R-1+pa+i, S-1+pb+j]
-> 9 channel-contraction taps total across the 4 phases (vs 36 for the
fully-composed kernel). Weights are pre-scaled by 1/16 (the FIR per-axis
1/4 gains) and bias/64 is folded in during the PSUM->SBUF drain (ACT),
which also casts to bf16. Phase tiles Yp[pa] are [128, 66, 132] with the
two column phases packed side by side and a bias/64 pad frame so the FIR
boundary handling is exact.

Stage 2 (GpSimd + DVE): the 4x4 FIR = outer((1,3,3,1),(1,3,3,1))/16 on the
2x-upsampled grid, evaluated as three box-filter adds per axis directly in
phase space (bf16, DVE 2x mode), in chunks of 16 output rows:
  C1[m] = y[m] + y[m+1]; C2[m] = C1[m] + C1[m+1]; V[A] = C2[A-1] + C2[A]
then the same cascade over columns; the final add writes fp32 directly
into the interleaved output staging tile.

Issue order: stage-1 iterates row-groups outer / phases inner (edge strips
first) so stage-2 chunks become runnable early; stage-2(cob=0) is emitted
interleaved with stage-1(cob=1) to keep all engines busy.
"""

import json

import numpy as np

import concourse.bass as bass
import concourse.mybir as mybir
import concourse.tile as tile
from concourse.bass_utils import run_bass_kernel_spmd

# ---------------------------------------------------------------------------
# BIR post-pass: this walrus build rejects instructions carrying more than one
# sem wait (e.g. Tile's kernel-tail Drain gets 3). Hoist extras into
# standalone EventSemaphore instructions right before the owner.
# ---------------------------------------------------------------------------
_MAX_WAITS = 1


def _split_waits(j: dict) -> dict:
    for fn in j.get("functions", []):
        for blk in fn.get("blocks", []):
            insts = blk.get("instructions")
            if not insts:
                continue
            out = []
            for inst in insts:
                si = inst.get("sync_info") or {}
                waits = si.get("on_wait") or []
                if len(waits) > _MAX_WAITS:
                    for k, w in enumerate(waits[_MAX_WAITS:]):
                        out.append(
                            {
                                "debug": inst.get("debug", 0),
                                "engine": inst["engine"],
                                "ins": [],
                                "name": f"{inst['name']}-wsplit{k}",
                                "opcode": "EventSemaphore",
                                "outs": [],
                                "sync_info": {"on_update": [], "on_wait": [w]},
                            }
                        )
                    si["on_wait"] = waits[:_MAX_WAITS]
                out.append(inst)
            blk["instructions"] = out
    return j


_orig_to_json_bytes = bass.Bass.to_json_bytes


def _patched_to_json_bytes(self):
    return json.dumps(_split_waits(json.loads(_orig_to_json_bytes(self)))).encode()


bass.Bass.to_json_bytes = _patched_to_json_bytes

# ---------------------------------------------------------------------------
# Problem constants (hardcoded; kernel.py must be self-contained)
# ---------------------------------------------------------------------------
N, C, H, W = 8, 256, 64, 64
OH, OW = 2 * H, 2 * W
N_CORES = 8
F32 = mybir.dt.float32
F32R = mybir.dt.float32r
BF16 = mybir.dt.bfloat16
IDENT = mybir.ActivationFunctionType.Identity

_PHASES = [(0, 0), (0, 1), (1, 0), (1, 1)]


def _taps(pa, pb):
    ii = (0, 1) if pa == 0 else (0,)
    jj = (0, 1) if pb == 0 else (0,)
    return [(i, j) for i in ii for j in jj]


_WBLOCKS = []
for pa, pb in _PHASES:
    for i, j in _taps(pa, pb):
        for cib in range(2):
            for cob in range(2):
                _WBLOCKS.append((pa, pb, i, j, cib, cob))
_WIDX = {k: n for n, k in enumerate(_WBLOCKS)}
NW = len(_WBLOCKS)  # 36


def _stage1_weights(w: np.ndarray) -> np.ndarray:
    """[256,256,3,3] -> lhsT [128 ci, NW, 128 co], scaled by 1/16."""
    Wm = np.zeros((128, NW, 128), dtype=np.float32)
    for n, (pa, pb, i, j, cib, cob) in enumerate(_WBLOCKS):
        blk = w[
            cob * 128 : (cob + 1) * 128, cib * 128 : (cib + 1) * 128, pa + 2 * i, pb + 2 * j
        ]  # [co, ci]
        Wm[:, n, :] = blk.T / 16.0
    return Wm


def build_nc(reps: int = 1) -> bass.Bass:
    nc = bass.Bass("TRN2", target_bir_lowering=False, debug=False)
    x_d = nc.dram_tensor("x", [C, H + 2, W + 2], BF16, kind="ExternalInput").ap()
    w_d = nc.dram_tensor("w", [128, NW * 128], BF16, kind="ExternalInput").ap()
    b_d = nc.dram_tensor("bias", [2, 128], F32, kind="ExternalInput").ap()
    out_d = nc.dram_tensor("out", [C, OH, OW], F32, kind="ExternalOutput").ap()

    xb = x_d.rearrange("(b p) h w -> b p h w", p=128)
    wb = w_d.rearrange("p (a b) -> p a b", b=128)

    with tile.TileContext(nc) as tc:
        with (
            tc.tile_pool(name="const", bufs=1) as cpool,
            tc.tile_pool(name="ypers", bufs=1) as ypool,
            tc.tile_pool(name="psum", bufs=3, space="PSUM") as ppool,
            tc.tile_pool(name="pedge", bufs=2, space="PSUM") as epool,
            tc.tile_pool(name="s2", bufs=3) as spool,
            tc.tile_pool(name="outs", bufs=3) as opool,
        ):
            # split input DMAs into bands so PE can start early
            wt = cpool.tile([128, NW, 128], BF16)
            for h in range(2):
                nc.sync.dma_start(wt[:, h * 18 : h * 18 + 18, :], wb[:, h * 18 : h * 18 + 18, :])
            bt = cpool.tile([128, 2], F32)
            nc.sync.dma_start(bt[:], b_d.rearrange("b p -> p b"))
            zt = cpool.tile([128, 132], F32)
            nc.vector.memset(zt[:], 0.0)

            xpad = [cpool.tile([128, H + 2, W + 2], BF16, name=f"xp{i}") for i in range(2)]
            for cib in range(2):
                for r0, r1 in ((0, 24), (24, 48), (48, 66)):
                    nc.sync.dma_start(
                        xpad[cib][:, r0:r1, :], xb[cib][:, r0:r1, :]
                    )

            # persistent Y phase tiles, frames pre-filled with bias/64
            Yp = {}
            for cob in range(2):
                for pa in range(2):
                    t = ypool.tile([128, 66, 132], BF16, name=f"Y{cob}{pa}")
                    Yp[(cob, pa)] = t
                    bias_ap = bt[:, cob : cob + 1]
                    frame_rows = [65] if pa == 0 else [0, 65]
                    for fr in frame_rows:
                        nc.scalar.activation(
                            t[:, fr, :], zt[:], IDENT, bias=bias_ap, scale=1.0
                        )
                    for fc in (65, 66, 131):
                        nc.scalar.activation(
                            t[:, :, fc], zt[:, 0:66], IDENT, bias=bias_ap, scale=1.0
                        )

            def s1_edges(cob):
                """Edge col strips (S=64 for pb=0 phases) + row remainders
                (R=64 for pa=0 phases), all accumulated in one psum bank."""
                bias_ap = bt[:, cob : cob + 1]
                pe = epool.tile([128, 512], F32, tag="pe", name="pe")
                off = 0
                drains = []
                for pa, pb in _PHASES:
                    taps = _taps(pa, pb)
                    nR = 65 if pa == 0 else 64
                    t0 = 0 if pa == 0 else 1
                    u0 = 0 if pb == 0 else 67
                    yt = Yp[(cob, pa)]
                    acc = [(i, j, cib) for (i, j) in taps for cib in range(2)]
                    if pb == 0:  # col strip S=64, rows 0..nR-1
                        for st, (i, j, cib) in enumerate(acc):
                            lhsT = wt[:, _WIDX[(pa, pb, i, j, cib, cob)], :]
                            rhs = xpad[cib][:, pa + i : pa + i + nR, 64 + pb + j]
                            nc.tensor.matmul(
                                pe[:, off : off + nR],
                                lhsT,
                                rhs,
                                start=(st == 0),
                                stop=(st == len(acc) - 1),
                            )
                        drains.append((yt[:, t0 : t0 + nR, u0 + 64], pe[:, off : off + nR]))
                        off += nR
                    if pa == 0:  # row remainder R=64, cols 0..63
                        for st, (i, j, cib) in enumerate(acc):
                            lhsT = wt[:, _WIDX[(pa, pb, i, j, cib, cob)], :]
                            rhs = xpad[cib][:, 64 + pa + i, pb + j : pb + j + 64]
                            nc.tensor.matmul(
                                pe[:, off : off + 64],
                                lhsT,
                                rhs,
                                start=(st == 0),
                                stop=(st == len(acc) - 1),
                            )
                        drains.append((yt[:, t0 + 64, u0 : u0 + 64], pe[:, off : off + 64]))
                        off += 64
                for dst, src in drains:
                    nc.scalar.activation(dst, src, IDENT, bias=bias_ap, scale=1.0)

            def s1_rowgroup(cob, rg):
                """Main-grid rows rg*16..rg*16+15, cols 0..63, all 4 phases."""
                bias_ap = bt[:, cob : cob + 1]
                R0 = rg * 16
                for pa, pb in _PHASES:
                    taps = _taps(pa, pb)
                    t0 = 0 if pa == 0 else 1
                    u0 = 0 if pb == 0 else 67
                    yt = Yp[(cob, pa)]
                    ps = ppool.tile([128, 16, 64], F32, tag="ps", name="ps")
                    for sub in range(2):
                        Rs = R0 + sub * 8
                        acc = [(i, j, cib) for (i, j) in taps for cib in range(2)]
                        for st, (i, j, cib) in enumerate(acc):
                            lhsT = wt[:, _WIDX[(pa, pb, i, j, cib, cob)], :]
                            rhs = xpad[cib][
                                :, Rs + pa + i : Rs + pa + i + 8, pb + j : pb + j + 64
                            ]
                            nc.tensor.matmul(
                                ps[:, sub * 8 : sub * 8 + 8, :],
                                lhsT,
                                rhs,
                                start=(st == 0),
                                stop=(st == len(acc) - 1),
                            )
                    nc.scalar.activation(
                        yt[:, t0 + R0 : t0 + R0 + 16, u0 : u0 + 64],
                        ps[:],
                        IDENT,
                        bias=bias_ap,
                        scale=1.0,
                    )

            def s2_chunk(cob, k):
                """16 output rows 16k..16k+15."""
                y0 = Yp[(cob, 0)]
                y1 = Yp[(cob, 1)]
                r = 8 * k
                c1e = spool.tile([128, 9, 132], BF16, tag="c1e", name="c1e")
                c1o = spool.tile([128, 9, 132], BF16, tag="c1o", name="c1o")
                nc.vector.tensor_add(c1e[:], y0[:, r : r + 9, :], y1[:, r + 1 : r + 10, :])
                nc.vector.tensor_add(c1o[:], y1[:, r : r + 9, :], y0[:, r : r + 9, :])
                c2e = spool.tile([128, 8, 132], BF16, tag="c2e", name="c2e")
                c2o = spool.tile([128, 9, 132], BF16, tag="c2o", name="c2o")
                nc.vector.tensor_add(c2e[:], c1e[:, 0:8, :], c1o[:, 1:9, :])
                nc.vector.tensor_add(c2o[:], c1o[:], c1e[:])
                # V packed: rows 0..7 = even out rows (ra=0), 8..15 = odd
                vv = spool.tile([128, 16, 132], BF16, tag="vv", name="vv")
                nc.vector.tensor_add(vv[:, 0:8, :], c2o[:, 0:8, :], c2e[:])
                nc.vector.tensor_add(vv[:, 8:16, :], c2e[:], c2o[:, 1:9, :])
                d1e = spool.tile([128, 16, 65], BF16, tag="d1e", name="d1e")
                d1o = spool.tile([128, 16, 65], BF16, tag="d1o", name="d1o")
                nc.vector.tensor_add(d1e[:], vv[:, :, 0:65], vv[:, :, 67:132])
                nc.vector.tensor_add(d1o[:], vv[:, :, 66:131], vv[:, :, 0:65])
                d2e = spool.tile([128, 16, 64], BF16, tag="d2e", name="d2e")
                d2o = spool.tile([128, 16, 65], BF16, tag="d2o", name="d2o")
                nc.vector.tensor_add(d2e[:], d1e[:, :, 0:64], d1o[:, :, 1:65])
                nc.vector.tensor_add(d2o[:], d1o[:], d1e[:])
                # Finals on GpSimd, writing fp32 interleaved straight into the
                # DMA staging tile: keeps the slow engine off the DVE critical
                # path (tail work feeding the DMA only).
                og = opool.tile([128, 16, 128], F32, tag="og", name="og")
                ov = og.rearrange("p (q a) (s b) -> p q a s b", a=2, b=2)
                for rb in range(2):
                    # in-order dims (ra, q, S) -> out AP [2ra, 8q, 64S]
                    dsrc0 = d2o[:, :, 0:64] if rb == 0 else d2e[:, :, :]
                    dsrc1 = d2e[:, :, :] if rb == 0 else d2o[:, :, 1:65]
                    dst_ap = ov.rearrange("p q a s b -> p a q s b")[:, :, :, :, rb]
                    nc.gpsimd.tensor_add(
                        dst_ap,
                        dsrc0.rearrange("p (a q) s -> p a q s", a=2),
                        dsrc1.rearrange("p (a q) s -> p a q s", a=2),
                    )
                dst = out_d[cob * 128 : (cob + 1) * 128, 16 * k : 16 * k + 16, :]
                nc.sync.dma_start(dst, og[:])

            def body():
                # interleave: emit each stage-2 chunk right after the last
                # stage-1 row-group its Y rows depend on has been issued.
                for cob in range(2):
                    s1_edges(cob)
                    s1_rowgroup(cob, 0)
                    s1_rowgroup(cob, 1)
                    s2_chunk(cob, 0)
                    s2_chunk(cob, 1)
                    s2_chunk(cob, 2)
                    s1_rowgroup(cob, 2)
                    s2_chunk(cob, 3)
                    s2_chunk(cob, 4)
                    s1_rowgroup(cob, 3)
                    s2_chunk(cob, 5)
                    s2_chunk(cob, 6)
                    s2_chunk(cob, 7)

            if reps == 1:
                body()
            else:
                with tc.For_i(0, reps):
                    body()
    return nc


_CACHED_NC = {}


def _get_nc(reps: int = 1) -> bass.Bass:
    if reps not in _CACHED_NC:
        _CACHED_NC[reps] = build_nc(reps)
    return _CACHED_NC[reps]


def _prep(x, weight, bias):
    import ml_dtypes

    Wm = _stage1_weights(np.asarray(weight, dtype=np.float32))
    b2 = np.ascontiguousarray(
        (np.asarray(bias, dtype=np.float32) / 64.0).reshape(2, 128)
    )
    xs = np.pad(
        np.asarray(x, dtype=np.float32), ((0, 0), (0, 0), (1, 1), (1, 1))
    )
    return (
        xs.astype(ml_dtypes.bfloat16),
        Wm.reshape(128, -1).astype(ml_dtypes.bfloat16),
        b2,
    )


def _run(x, weight, bias, reps: int = 1):
    xs, Wm, b2 = _prep(x, weight, bias)
    nc = _get_nc(reps)
    in_maps = [{"x": xs[i], "w": Wm, "bias": b2} for i in range(N_CORES)]
    res = run_bass_kernel_spmd(nc, in_maps, list(range(N_CORES)))
    return np.stack([res.results[i]["out"] for i in range(N_CORES)])


def kernel(x, weight, bias):
    return _run(x, weight, bias, reps=1)
